# revision 1
# baseline (speedup 1.0000x reference)
"""Pairwise squared Euclidean distance on Trainium2, sharded over 8 NeuronCores.

dist[i, j] = ||s_i - t_j||^2 = s_sq[i] + t_sq[j] - 2 * (s @ t.T)[i, j]

Sharding: rows of s (and of the output) are split across the 8 cores;
t is replicated to every core. Each core computes a [2048, 16384] tile.

Per-core device program (bf16 split-precision path):
  The fp32 cross term is computed as two full-rate bf16 matmuls with fp32
  PSUM accumulation. With S = -2*s^T split as S ~ sh + sl (bf16 hi/lo) and
  T = t^T split as T ~ th + tl:
      -2*s@t.T ~ [sh; sl]^T @ [th; th]  (K=128, matmul 1)
               + [sh]^T @ [tl]          (folded into matmul 2)
  The dropped sl*tl term is ~2^-17 relative. Matmul 2 (K=97) also carries
  all-ones lhsT rows at partitions 64/96 against bf16 hi/lo rows of t_sq
  (rows 65..95 are zero padding: engine APs must start on partition
  0/32/64/96), so PSUM ends up holding t_sq[j] - 2*cross[i, j]. The
  per-partition s_sq[i] (exact fp32, from ACT Square with free-dim
  accumulation) is added during the PSUM->SBUF copy (ACT bias / DVE
  tensor_scalar), and staging groups are DMA'd to the output.

  Transposes are PE matmuls against identity / -2*identity. Data that must
  land on partitions 64+ (the sl rows of A, the th duplicate rows of B1)
  moves via SBUF->SBUF DMA on the ACT HWDGE ring, which keeps the SP ring
  free for input/output traffic and costs ACT no stall (the data it waits
  on is produced by earlier ACT ops). t_sq is reduced over d with an
  all-ones [64, 128] stationary operand, which replicates it onto every
  PSUM partition so rows 64/96 can be copied partition-aligned.

  t-prep is chunked (2048 columns) and the main loop is grouped (4096
  columns); Tile's range-accurate dependency tracking lets group g's
  matmuls and output DMAs overlap with the prep of later chunks.
"""

import numpy as np

import concourse.mybir as mybir
import concourse.tile as tile
from concourse import bacc
from concourse.masks import make_identity

F32 = mybir.dt.float32
BF16 = mybir.dt.bfloat16

N_CORES = 8
N, Q, D = 16384, 16384, 64
N_SHARD = N // N_CORES  # 2048


def build_nc(n_rows=N_SHARD, q=Q, d=D, chunk=2048, gcols=4096):
    assert n_rows % 128 == 0 and q % gcols == 0 and gcols % chunk == 0
    assert chunk % 512 == 0 and d == 64
    m_tiles = n_rows // 128
    n_chunks = q // chunk
    n_groups = q // gcols
    t_per_chunk = chunk // 128

    nc = bacc.Bacc()
    s = nc.dram_tensor("s", [n_rows, d], F32, kind="ExternalInput")
    t = nc.dram_tensor("t", [q, d], F32, kind="ExternalInput")
    o = nc.dram_tensor("o", [n_rows, q], F32, kind="ExternalOutput")

    with tile.TileContext(nc) as tc:
        with (
            tc.tile_pool(name="const", bufs=1) as const,
            tc.tile_pool(name="work", bufs=16) as work,
            tc.tile_pool(name="chunks", bufs=2) as chunks,
            tc.tile_pool(name="stage", bufs=3) as stage,
            tc.tile_pool(name="psum_prep", bufs=2, space="PSUM") as psum_prep,
            tc.tile_pool(name="psum_mm", bufs=4, space="PSUM") as psum_mm,
        ):
            identity = const.tile([128, 128], F32, name="identity")
            make_identity(nc, identity)
            neg2I = const.tile([128, 128], F32, name="neg2I")
            make_identity(nc, neg2I)
            nc.scalar.mul(neg2I, neg2I, -2.0)
            ones_mat = const.tile([d, 128], F32, name="ones_mat")
            nc.vector.memset(ones_mat, 1.0)

            # PE warmup: ~7us of dense fp32 matmuls to trip the HAM clock
            # gate from 4/8 (1.2 GHz) to 8/8 (2.4 GHz) early. The tiny
            # DMA to o[0:1, 0:1] keeps the chain live through DCE; the
            # real output of that region is written later (WAW-ordered).
            pw = psum_prep.tile([128, 128], F32, name="pw", tag="ps")
            for _ in range(16):
                nc.tensor.matmul(pw, identity, identity, start=True, stop=True)
            warm_sb = const.tile([1, 1], F32, name="warm_sb")
            nc.scalar.copy(warm_sb, pw[0:1, 0:1])
            nc.sync.dma_start(out=o[0:1, 0:1], in_=warm_sb)

            K2 = 97
            A = const.tile([128, n_rows], BF16, name="A")     # sh / sl
            A2 = const.tile([K2, n_rows], BF16, name="A2")    # sh / ones+zeros
            B1 = const.tile([128, q], BF16, name="B1")        # th / th
            B2 = const.tile([K2, q], BF16, name="B2")         # tl / tsq hi,lo
            slb = const.tile([64, n_rows], BF16, name="slb")  # sl staging
            s_sq = const.tile([128, m_tiles], F32, name="s_sq")
            nc.gpsimd.memset(A2[64:96, :], 0.0)
            nc.vector.memset(A2[64:65, :], 1.0)
            nc.vector.memset(A2[96:97, :], 1.0)
            nc.gpsimd.memset(B2[64:96, :], 0.0)

            # ---- s prep: sh, sl, s_sq ----
            for m in range(m_tiles):
                rows = slice(m * 128, (m + 1) * 128)
                sn = work.tile([128, d], F32, name="sn", tag="sn")
                nc.sync.dma_start(out=sn, in_=s[rows, :])
                pss = psum_prep.tile([d, 128], F32, name="pss", tag="ps")
                # regular matmul vs -2*I: pss = sn.T @ (-2 I) = -2 s^T (exact)
                nc.tensor.matmul(pss, sn, neg2I, start=True, stop=True)
                nc.scalar.copy(A[0:d, rows], pss)          # sh
                nc.vector.tensor_sub(slb[:, rows], pss, A[0:d, rows])  # sl
                # sh copy for mm2, read from SBUF so it doesn't hold the bank
                nc.vector.tensor_copy(A2[0:d, rows], A[0:d, rows])
                sqs = work.tile([128, d], F32, name="sqs", tag="sqs")
                nc.scalar.activation(
                    sqs,
                    sn,
                    func=mybir.ActivationFunctionType.Square,
                    accum_out=s_sq[:, m : m + 1],
                )
            # move sl onto partitions 64..127 (ACT HWDGE ring)
            nc.scalar.dma_start(out=A[64:128, :], in_=slb[:, :])

            # ---- t prep: th, tl, t_sq (chunked) ----
            for ch in range(n_chunks):
                ccols = slice(ch * chunk, (ch + 1) * chunk)
                tTf = chunks.tile([d, chunk], F32, name="tTf", tag="tTf")
                for j in range(t_per_chunk):
                    k = ch * t_per_chunk + j
                    tn = work.tile([128, d], F32, name="tn", tag="tn")
                    nc.sync.dma_start(out=tn, in_=t[k * 128 : (k + 1) * 128, :])
                    pst = psum_prep.tile([d, 128], F32, name="pst", tag="pp")
                    nc.tensor.transpose(pst, tn, identity)
                    dst = tTf[:, j * 128 : (j + 1) * 128]
                    if k % 2 == 0:
                        nc.scalar.copy(dst, pst)
                    else:
                        nc.vector.tensor_copy(dst, pst)
                # th = bf16(T);  tl = bf16(T - th)
                nc.scalar.copy(B1[0:d, ccols], tTf)
                nc.scalar.dma_start(out=B1[64:128, ccols], in_=B1[0:d, ccols])
                nc.vector.tensor_sub(B2[0:d, ccols], tTf, B1[0:d, ccols])
                # t_sq = ones^T @ (T * T), replicated over all out partitions
                sqf = chunks.tile([d, chunk], F32, name="sqf", tag="sqf")
                nc.scalar.square(sqf, tTf)
                for i in range(chunk // 512):
                    cols = slice(ch * chunk + i * 512, ch * chunk + (i + 1) * 512)
                    pts = psum_prep.tile([128, 512], F32, name="pts", tag="pp")
                    nc.tensor.matmul(
                        pts, ones_mat, sqf[:, i * 512 : (i + 1) * 512],
                        start=True, stop=True,
                    )
                    # row 64 <- hi = bf16(t_sq); row 96 <- lo = bf16(t_sq - hi)
                    nc.scalar.copy(B2[64:65, cols], pts[64:65, :])
                    nc.scalar.copy(B2[96:97, cols], pts[96:97, :])
                    nc.vector.tensor_sub(
                        B2[96:97, cols], pts[96:97, :], B2[96:97, cols]
                    )

            # ---- main loop, grouped over output columns ----
            for g in range(n_groups):
                for m in range(m_tiles):
                    rows = slice(m * 128, (m + 1) * 128)
                    stg = stage.tile([128, gcols], F32, name="stg", tag="stg")
                    for ci in range(gcols // 512):
                        c = (g * gcols) // 512 + ci
                        cols = slice(c * 512, (c + 1) * 512)
                        ps = psum_mm.tile([128, 512], F32, name="ps", tag="mm")
                        nc.tensor.matmul(
                            ps, A[:, rows], B1[:, cols], start=True, stop=False
                        )
                        nc.tensor.matmul(
                            ps, A2[:, rows], B2[:, cols], start=False, stop=True
                        )
                        dst = stg[:, ci * 512 : (ci + 1) * 512]
                        if ci % 2 == 0:
                            nc.scalar.add(dst, ps, s_sq[:, m : m + 1])
                        else:
                            nc.vector.tensor_scalar_add(dst, ps, s_sq[:, m : m + 1])
                    # alternate the two HWDGE rings (SP / ACT) for 2x the
                    # DMA packet-processing throughput on the output stream
                    out_eng = nc.sync if m % 2 == 0 else nc.scalar
                    out_eng.dma_start(
                        out=o[rows, g * gcols : (g + 1) * gcols], in_=stg
                    )

    nc.finalize()
    return nc


_NC_CACHE = {}


def _get_nc(key=None):
    if key is None:
        key = (N_SHARD, Q, D)
    if key not in _NC_CACHE:
        _NC_CACHE[key] = build_nc(*key)
    return _NC_CACHE[key]


def make_in_maps(inputs):
    s = np.asarray(inputs["s"], dtype=np.float32)
    t = np.asarray(inputs["t"], dtype=np.float32)
    assert s.shape == (N, D) and t.shape == (Q, D), (s.shape, t.shape)
    return [{"s": s[c * N_SHARD : (c + 1) * N_SHARD], "t": t} for c in range(N_CORES)]


def _run(inputs, **spmd_kwargs):
    from concourse.bass_utils import run_bass_kernel_spmd

    nc = _get_nc()
    in_maps = make_in_maps(inputs)
    res = run_bass_kernel_spmd(nc, in_maps, list(range(N_CORES)), **spmd_kwargs)
    out = np.concatenate([res.results[c]["o"] for c in range(N_CORES)], axis=0)
    return out, res


def kernel(**inputs):
    out, _ = _run(inputs)
    return out



# revision 2
# speedup vs baseline: 1.1301x; 1.1301x over previous
"""Pairwise squared Euclidean distance on Trainium2, sharded over 8 NeuronCores.

dist[i, j] = ||s_i - t_j||^2 = s_sq[i] + t_sq[j] - 2 * (s @ t.T)[i, j]

Sharding: rows of s (and of the output) are split across the 8 cores;
t is replicated to every core. Each core computes a [2048, 16384] tile.

Per-core device program (single-matmul bf16 path):
  The tolerance (rel 2e-2) allows computing the cross term from the bf16
  hi parts alone: with A = bf16(-2*s)^T and B = bf16(t)^T, a single K=65
  matmul per output tile produces  -2*s@t.T + t_sq  in fp32 PSUM (row 64
  of A is all-ones against row 64 of B holding bf16(t_sq), computed on PE
  from an all-ones stationary operand over square(B)).  Measured rel err
  of this scheme is ~1.9e-3.  The exact fp32 per-partition s_sq[i] (ACT
  Square with free-dim accumulation) is added during the PSUM->SBUF copy
  (ACT bias / DVE tensor_scalar), and staging tiles are DMA'd to the
  output on alternating HWDGE rings (SP / ACT).

  Inputs are fetched with ONE strided DMA per 2048-row block (3D access
  pattern, 256B innermost runs) instead of 16 small loads -- the
  per-DMA sequencer cost (~600ns) and HWDGE slot (~630ns), not bytes,
  dominated the old prep phase.  Transposes are PE matmuls against
  identity / -2*identity from [128, 64] column views of the block.

  t-prep is chunked (2048 columns) and emitted interleaved with the main
  loop at half-group granularity (software pipeline one chunk ahead), so
  the output DMA stream starts after only chunk 0/1 and never starves.
"""

import numpy as np

import concourse.mybir as mybir
import concourse.tile as tile
from concourse import bacc
from concourse.masks import make_identity

F32 = mybir.dt.float32
BF16 = mybir.dt.bfloat16

N_CORES = 8
N, Q, D = 16384, 16384, 64
N_SHARD = N // N_CORES  # 2048


def build_nc(n_rows=N_SHARD, q=Q, d=D, chunk=2048, gcols=4096):
    assert n_rows % 128 == 0 and q % gcols == 0 and gcols % chunk == 0
    assert chunk % 512 == 0 and d == 64
    m_tiles = n_rows // 128          # 16
    n_chunks = q // chunk            # 8
    n_groups = q // gcols            # 4
    ch_per_group = gcols // chunk    # 2
    t_per_chunk = chunk // 128       # 16
    K = d + 1                        # 65: d rows of sh, row 64 = ones / t_sq

    nc = bacc.Bacc()
    s = nc.dram_tensor("s", [n_rows, d], F32, kind="ExternalInput")
    t = nc.dram_tensor("t", [q, d], F32, kind="ExternalInput")
    o = nc.dram_tensor("o", [n_rows, q], F32, kind="ExternalOutput")

    with tile.TileContext(nc) as tc:
        with (
            tc.tile_pool(name="const", bufs=1) as const,
            tc.tile_pool(name="work", bufs=4) as work,
            tc.tile_pool(name="chunks", bufs=2) as chunks,
            tc.tile_pool(name="stage", bufs=3) as stage,
            tc.tile_pool(name="psum_prep", bufs=2, space="PSUM") as psum_prep,
            tc.tile_pool(name="psum_mm", bufs=4, space="PSUM") as psum_mm,
        ):
            identity = const.tile([128, 128], F32, name="identity")
            make_identity(nc, identity)
            neg2I = const.tile([128, 128], F32, name="neg2I")
            make_identity(nc, neg2I)
            nc.scalar.mul(neg2I, neg2I, -2.0)
            ones_mat = const.tile([d, 128], F32, name="ones_mat")
            nc.vector.memset(ones_mat, 1.0)

            # PE warmup: dense fp32 matmuls to trip the HAM clock gate from
            # 4/8 (1.2 GHz) to 8/8 (2.4 GHz) early. The tiny DMA to
            # o[0:1, 0:1] keeps the chain live through DCE; the real output
            # of that region is written later (WAW-ordered).
            pw = psum_prep.tile([128, 128], F32, name="pw", tag="pp")
            for _ in range(16):
                nc.tensor.matmul(pw, identity, identity, start=True, stop=True)
            warm_sb = const.tile([1, 1], F32, name="warm_sb")
            nc.scalar.copy(warm_sb, pw[0:1, 0:1])
            nc.sync.dma_start(out=o[0:1, 0:1], in_=warm_sb)

            A = const.tile([K, n_rows], BF16, name="A")   # sh rows 0..63, 64=ones
            B = const.tile([K, q], BF16, name="B")        # th rows 0..63, 64=t_sq
            s_sq = const.tile([128, m_tiles], F32, name="s_sq")
            nc.vector.memset(A[64:65, :], 1.0)

            # ---- s prep: one strided load, then per-tile transpose+square ----
            S = const.tile([128, m_tiles * d], F32, name="S")
            nc.sync.dma_start(
                out=S[:, :].rearrange("p (m d) -> p m d", m=m_tiles, d=d),
                in_=s[:, :].rearrange("(m p) d -> p m d", m=m_tiles, p=128),
            )
            for m in range(m_tiles):
                V = S[:, m * d : (m + 1) * d]
                pss = psum_prep.tile([d, 128], F32, name="pss", tag="pp")
                # pss = V.T @ (-2 I) = -2 s^T  (exact)
                nc.tensor.matmul(pss, V, neg2I, start=True, stop=True)
                dst = A[0:d, m * 128 : (m + 1) * 128]
                if m % 2 == 0:
                    nc.scalar.copy(dst, pss)
                else:
                    nc.vector.tensor_copy(dst, pss)
                sqs = work.tile([128, d], F32, name="sqs", tag="sqs")
                nc.scalar.activation(
                    sqs,
                    V,
                    func=mybir.ActivationFunctionType.Square,
                    accum_out=s_sq[:, m : m + 1],
                )

            # ---- t prep, one 2048-column chunk at a time ----
            def prep_chunk(ch):
                base = ch * chunk
                G = chunks.tile([128, t_per_chunk * d], F32, name="G", tag="G")
                nc.sync.dma_start(
                    out=G[:, :].rearrange("p (j d) -> p j d", j=t_per_chunk, d=d),
                    in_=t[base : base + chunk, :].rearrange(
                        "(j p) d -> p j d", j=t_per_chunk, p=128
                    ),
                )
                for j in range(t_per_chunk):
                    V = G[:, j * d : (j + 1) * d]
                    pst = psum_prep.tile([d, 128], F32, name="pst", tag="pp")
                    nc.tensor.transpose(pst, V, identity)
                    dst = B[0:d, base + j * 128 : base + (j + 1) * 128]
                    if j % 2 == 0:
                        nc.scalar.copy(dst, pst)
                    else:
                        nc.vector.tensor_copy(dst, pst)
                # t_sq = ones^T @ square(th), replicated over all partitions
                sq = chunks.tile([d, chunk], F32, name="sq", tag="sq")
                nc.scalar.square(sq, B[0:d, base : base + chunk])
                for i in range(chunk // 512):
                    cols = slice(base + i * 512, base + (i + 1) * 512)
                    pts = psum_prep.tile([128, 512], F32, name="pts", tag="pp")
                    nc.tensor.matmul(
                        pts, ones_mat, sq[:, i * 512 : (i + 1) * 512],
                        start=True, stop=True,
                    )
                    if i % 2 == 0:
                        nc.scalar.copy(B[64:65, cols], pts[64:65, :])
                    else:
                        nc.vector.tensor_copy(B[64:65, cols], pts[64:65, :])

            # ---- main loop tile ----
            def main_tile(g, m):
                rows = slice(m * 128, (m + 1) * 128)
                stg = stage.tile([128, gcols], F32, name="stg", tag="stg")
                for ci in range(gcols // 512):
                    c = (g * gcols) // 512 + ci
                    cols = slice(c * 512, (c + 1) * 512)
                    ps = psum_mm.tile([128, 512], F32, name="ps", tag="mm")
                    nc.tensor.matmul(
                        ps, A[:, rows], B[:, cols], start=True, stop=True
                    )
                    dst = stg[:, ci * 512 : (ci + 1) * 512]
                    if ci % 2 == 0:
                        nc.scalar.add(dst, ps, s_sq[:, m : m + 1])
                    else:
                        nc.vector.tensor_scalar_add(dst, ps, s_sq[:, m : m + 1])
                # alternate the two HWDGE rings (SP / ACT) for 2x the
                # DMA packet-processing throughput on the output stream
                out_eng = nc.sync if m % 2 == 0 else nc.scalar
                out_eng.dma_start(
                    out=o[rows, g * gcols : (g + 1) * gcols], in_=stg
                )

            # software pipeline: chunks for group g+1 are emitted between
            # the two halves of group g's tiles
            for ch in range(ch_per_group):
                prep_chunk(ch)
            for g in range(n_groups):
                for half in range(2):
                    ch_next = (g + 1) * ch_per_group + half
                    if ch_next < n_chunks:
                        prep_chunk(ch_next)
                    for m in range(half * (m_tiles // 2),
                                   (half + 1) * (m_tiles // 2)):
                        main_tile(g, m)

    nc.finalize()
    return nc


_NC_CACHE = {}


def _get_nc(key=None):
    if key is None:
        key = (N_SHARD, Q, D)
    if key not in _NC_CACHE:
        _NC_CACHE[key] = build_nc(*key)
    return _NC_CACHE[key]


def make_in_maps(inputs):
    s = np.asarray(inputs["s"], dtype=np.float32)
    t = np.asarray(inputs["t"], dtype=np.float32)
    assert s.shape == (N, D) and t.shape == (Q, D), (s.shape, t.shape)
    return [{"s": s[c * N_SHARD : (c + 1) * N_SHARD], "t": t} for c in range(N_CORES)]


def _run(inputs, **spmd_kwargs):
    from concourse.bass_utils import run_bass_kernel_spmd

    nc = _get_nc()
    in_maps = make_in_maps(inputs)
    res = run_bass_kernel_spmd(nc, in_maps, list(range(N_CORES)), **spmd_kwargs)
    out = np.concatenate([res.results[c]["o"] for c in range(N_CORES)], axis=0)
    return out, res


def kernel(**inputs):
    out, _ = _run(inputs)
    return out


# revision 9
# speedup vs baseline: 1.1875x; 1.0508x over previous
"""Pairwise squared Euclidean distance on Trainium2, sharded over 8 NeuronCores.

dist[i, j] = ||s_i - t_j||^2 = s_sq[i] + t_sq[j] - 2 * (s @ t.T)[i, j]

Sharding: rows of s (and of the output) are split across the 8 cores;
t is replicated to every core. Each core computes a [2048, 16384] tile.

Per-core device program (single-matmul bf16 path):
  The tolerance (rel 2e-2) allows computing the cross term from the bf16
  hi parts alone: with A = bf16(-2*s)^T and B = bf16(t)^T, a single K=65
  matmul per output tile produces  -2*s@t.T + t_sq  in fp32 PSUM (row 64
  of A is all-ones against row 64 of B holding bf16(t_sq), computed on PE
  from an all-ones stationary operand over square(B)).  Measured rel err
  of this scheme is ~1.9e-3.  The exact fp32 per-partition s_sq[i] (DVE
  fused square+reduce) is added during the PSUM->SBUF copy (ACT bias /
  DVE tensor_scalar), and staging tiles are DMA'd to the output on
  alternating HWDGE rings (SP / ACT).

  DMA traffic is minimized: t is fetched with ONE DMA per 2048-row chunk
  in a 16-rows-per-partition grouped layout whose innermost contiguous
  run is 4KB (full DMA bus rate; <512B runs pay a 2x penalty), then
  PE-transposed [128, 64] a column-view at a time; each transpose lands
  in B via a stride-16 free-dim scatter that reconstructs global column
  order.  s uses one strided 256B-run DMA (order must be preserved for
  the output rows, and it is 8x smaller than t).

  The main loop is emitted group-by-group (one 2048-column group per
  chunk); the load+prep of chunk g+2 is emitted AFTER group g's tiles so
  the first output DMA is gated only on chunk 0, while Tile's
  range-accurate dependency tracking overlaps later prep with the
  saturated output stream.
"""

import numpy as np

import concourse.mybir as mybir
import concourse.tile as tile
from concourse import bacc
from concourse.masks import make_identity

F32 = mybir.dt.float32
BF16 = mybir.dt.bfloat16

N_CORES = 8
N, Q, D = 16384, 16384, 64
N_SHARD = N // N_CORES  # 2048


def build_nc(n_rows=N_SHARD, q=Q, d=D, chunk=2048):
    assert n_rows % 128 == 0 and q % chunk == 0
    assert chunk % 512 == 0 and d == 64
    m_tiles = n_rows // 128          # 16
    n_chunks = q // chunk            # 8
    t_per_chunk = chunk // 128       # 16
    K = d + 1                        # 65: d rows of sh, row 64 = ones / t_sq

    nc = bacc.Bacc()
    s = nc.dram_tensor("s", [n_rows, d], F32, kind="ExternalInput")
    t = nc.dram_tensor("t", [q, d], F32, kind="ExternalInput")
    o = nc.dram_tensor("o", [n_rows, q], F32, kind="ExternalOutput")

    with tile.TileContext(nc) as tc:
        with (
            tc.tile_pool(name="const", bufs=1) as const,
            tc.tile_pool(name="work", bufs=4) as work,
            tc.tile_pool(name="chunks", bufs=4) as chunks,
            tc.tile_pool(name="stage", bufs=4) as stage,
            tc.tile_pool(name="psum_prep", bufs=4, space="PSUM") as psum_prep,
            tc.tile_pool(name="psum_mm", bufs=4, space="PSUM") as psum_mm,
        ):
            # s lands early: one strided DMA (3D AP, 256B runs) issued
            # before any const-init so nothing delays it
            S = const.tile([128, m_tiles * d], F32, name="S")
            nc.sync.dma_start(
                out=S[:, :].rearrange("p (m d) -> p m d", m=m_tiles, d=d),
                in_=s[:, :].rearrange("(m p) d -> p m d", m=m_tiles, p=128),
            )

            identity = const.tile([128, 128], F32, name="identity")
            make_identity(nc, identity)
            neg2I = const.tile([128, 128], F32, name="neg2I")
            make_identity(nc, neg2I)
            nc.scalar.mul(neg2I, neg2I, -2.0)
            ones_mat = const.tile([d, 128], F32, name="ones_mat")
            nc.vector.memset(ones_mat, 1.0)

            # PE warmup: dense fp32 matmuls to trip the HAM clock gate from
            # 4/8 (1.2 GHz) to 8/8 (2.4 GHz) early. The tiny DMA (on the
            # ACT ring, so the SP ring's input loads are not stalled) keeps
            # the chain live through DCE; the real output of that region is
            # written later (WAW-ordered).
            pw = psum_prep.tile([128, 128], F32, name="pw", tag="pp")
            for _ in range(16):
                nc.tensor.matmul(pw, identity, identity, start=True, stop=True)
            warm_sb = const.tile([1, 1], F32, name="warm_sb")
            nc.scalar.copy(warm_sb, pw[0:1, 0:1])
            nc.scalar.dma_start(out=o[0:1, 0:1], in_=warm_sb)

            A = const.tile([K, n_rows], BF16, name="A")   # sh rows 0..63, 64=ones
            B = const.tile([K, q], BF16, name="B")        # th rows 0..63, 64=t_sq
            s_sq = const.tile([128, m_tiles], F32, name="s_sq")
            nc.vector.memset(A[64:65, :], 1.0)

            # ---- s prep: per-tile transpose (PE) + fused square-reduce ----
            for m in range(m_tiles):
                V = S[:, m * d : (m + 1) * d]
                pss = psum_prep.tile([d, 128], F32, name="pss", tag="pp")
                # pss = V.T @ (-2 I) = -2 s^T  (exact)
                nc.tensor.matmul(pss, V, neg2I, start=True, stop=True)
                dst = A[0:d, m * 128 : (m + 1) * 128]
                if m % 2 == 0:
                    nc.scalar.copy(dst, pss)
                else:
                    nc.vector.tensor_copy(dst, pss)
                # exact fp32 row sums of s^2 (DVE fused square+accum)
                sqs = work.tile([128, d], F32, name="sqs", tag="sqs")
                nc.vector.tensor_tensor_reduce(
                    sqs,
                    V,
                    V,
                    1.0,
                    0.0,
                    mybir.AluOpType.mult,
                    mybir.AluOpType.add,
                    s_sq[:, m : m + 1],
                )

            # ---- t prep, one 2048-column chunk at a time ----
            def prep_chunk(ch):
                base = ch * chunk
                # grouped layout: partition p holds t rows base+16p..+15,
                # giving 4KB contiguous runs (full DMA bus rate)
                G = chunks.tile([128, t_per_chunk * d], F32, name="G", tag="G")
                nc.sync.dma_start(
                    out=G[:, :].rearrange("p (j d) -> p j d", j=t_per_chunk, d=d),
                    in_=t[base : base + chunk, :].rearrange(
                        "(p j) d -> p j d", p=128, j=t_per_chunk
                    ),
                )
                # B columns c = 16p + j: transpose view j, scatter stride 16
                Bv = B[0:d, base : base + chunk].rearrange(
                    "e (p j) -> e j p", p=128, j=t_per_chunk
                )
                for j in range(t_per_chunk):
                    V = G[:, j * d : (j + 1) * d]
                    pst = psum_prep.tile([d, 128], F32, name="pst", tag="pp")
                    nc.tensor.transpose(pst, V, identity)
                    if j % 2 == 0:
                        nc.scalar.copy(Bv[:, j, :], pst)
                    else:
                        nc.vector.tensor_copy(Bv[:, j, :], pst)
                # t_sq = ones^T @ square(th), replicated over all partitions
                sq = chunks.tile([d, chunk], F32, name="sq", tag="sq", bufs=2)
                nc.scalar.square(sq, B[0:d, base : base + chunk])
                for i in range(chunk // 512):
                    cols = slice(base + i * 512, base + (i + 1) * 512)
                    pts = psum_prep.tile([128, 512], F32, name="pts", tag="pp")
                    nc.tensor.matmul(
                        pts, ones_mat, sq[:, i * 512 : (i + 1) * 512],
                        start=True, stop=True,
                    )
                    if i % 2 == 0:
                        nc.scalar.copy(B[64:65, cols], pts[64:65, :])
                    else:
                        nc.vector.tensor_copy(B[64:65, cols], pts[64:65, :])

            # ---- main loop tile: one [128, chunk] staging tile ----
            def main_tile(g, m, split_out=False):
                rows = slice(m * 128, (m + 1) * 128)
                stg = stage.tile([128, chunk], F32, name="stg", tag="stg")
                for ci in range(chunk // 512):
                    c = (g * chunk) // 512 + ci
                    cols = slice(c * 512, (c + 1) * 512)
                    ps = psum_mm.tile([128, 512], F32, name="ps", tag="mm")
                    nc.tensor.matmul(
                        ps, A[:, rows], B[:, cols], start=True, stop=True
                    )
                    dst = stg[:, ci * 512 : (ci + 1) * 512]
                    if ci % 2 == 0:
                        nc.scalar.add(dst, ps, s_sq[:, m : m + 1])
                    else:
                        nc.vector.tensor_scalar_add(dst, ps, s_sq[:, m : m + 1])
                # alternate the two HWDGE rings (SP / ACT) for 2x the
                # DMA packet-processing throughput on the output stream
                out_eng = nc.sync if m % 2 == 0 else nc.scalar
                ocols = slice(g * chunk, (g + 1) * chunk)
                if not split_out:
                    out_eng.dma_start(out=o[rows, ocols], in_=stg)
                else:
                    # last tile of the kernel: drain in two halves on both
                    # rings so the tail transfer is half as long
                    h = chunk // 2
                    nc.sync.dma_start(
                        out=o[rows, g * chunk : g * chunk + h], in_=stg[:, 0:h]
                    )
                    nc.scalar.dma_start(
                        out=o[rows, g * chunk + h : (g + 1) * chunk],
                        in_=stg[:, h:chunk],
                    )

            # software pipeline: chunk g+2's load+prep is emitted after
            # group g's tiles (group g uses chunk g, one group per chunk)
            prep_chunk(0)
            prep_chunk(1)
            for g in range(n_chunks):
                for m in range(m_tiles):
                    last = g == n_chunks - 1 and m == m_tiles - 1
                    main_tile(g, m, split_out=last)
                if g + 2 < n_chunks:
                    prep_chunk(g + 2)

    nc.finalize()
    return nc


_NC_CACHE = {}


def _get_nc(key=None):
    if key is None:
        key = (N_SHARD, Q, D)
    if key not in _NC_CACHE:
        _NC_CACHE[key] = build_nc(*key)
    return _NC_CACHE[key]


def make_in_maps(inputs):
    s = np.asarray(inputs["s"], dtype=np.float32)
    t = np.asarray(inputs["t"], dtype=np.float32)
    assert s.shape == (N, D) and t.shape == (Q, D), (s.shape, t.shape)
    return [{"s": s[c * N_SHARD : (c + 1) * N_SHARD], "t": t} for c in range(N_CORES)]


def _run(inputs, **spmd_kwargs):
    from concourse.bass_utils import run_bass_kernel_spmd

    nc = _get_nc()
    in_maps = make_in_maps(inputs)
    res = run_bass_kernel_spmd(nc, in_maps, list(range(N_CORES)), **spmd_kwargs)
    out = np.concatenate([res.results[c]["o"] for c in range(N_CORES)], axis=0)
    return out, res


def kernel(**inputs):
    out, _ = _run(inputs)
    return out


# revision 33
# speedup vs baseline: 1.2238x; 1.0306x over previous
"""Pairwise squared Euclidean distance on Trainium2, sharded over 8 NeuronCores.

dist[i, j] = ||s_i - t_j||^2 = s_sq[i] + t_sq[j] - 2 * (s @ t.T)[i, j]

Sharding: rows of s (and of the output) are split across the 8 cores;
t is replicated to every core. Each core computes a [2048, 16384] tile.

Per-core device program (single-matmul bf16 path):
  The tolerance (rel 2e-2) allows computing the cross term from the bf16
  hi parts alone: with A = bf16(-2*s)^T and B = bf16(t)^T, a single K=65
  matmul per output tile produces  -2*s@t.T + t_sq  in fp32 PSUM (row 64
  of A is all-ones against row 64 of B holding bf16(t_sq), computed on PE
  from an all-ones stationary operand over square(B)).  Measured rel err
  of this scheme is ~1.9e-3.  The exact fp32 per-partition s_sq[i] (DVE
  fused square+reduce) is added during the PSUM->SBUF copy (ACT bias /
  DVE tensor_scalar), and staging tiles are DMA'd to the output on
  alternating HWDGE rings (SP / ACT).

  DMA traffic is minimized: t is fetched with ONE DMA per 2048-row chunk
  in a 16-rows-per-partition grouped layout whose innermost contiguous
  run is 4KB (full DMA bus rate; <512B runs pay a 2x penalty), then
  PE-transposed [128, 64] a column-view at a time; each transpose lands
  in B via a stride-16 free-dim scatter that reconstructs global column
  order.  s uses one strided 256B-run DMA (order must be preserved for
  the output rows, and it is 8x smaller than t).

  The main loop is emitted group-by-group (one 2048-column group per
  chunk); the load+prep of chunk g+2 is emitted AFTER group g's tiles so
  the first output DMA is gated only on chunk 0, while Tile's
  range-accurate dependency tracking overlaps later prep with the
  saturated output stream.
"""

import numpy as np

import concourse.mybir as mybir
import concourse.tile as tile
from concourse import bacc
from concourse.masks import make_identity

F32 = mybir.dt.float32
BF16 = mybir.dt.bfloat16

N_CORES = 8
N, Q, D = 16384, 16384, 64
N_SHARD = N // N_CORES  # 2048


def build_nc(n_rows=N_SHARD, q=Q, d=D, chunk=2048):
    assert n_rows % 128 == 0 and q % chunk == 0
    assert chunk % 512 == 0 and d == 64
    m_tiles = n_rows // 128          # 16
    n_chunks = q // chunk            # 8
    t_per_chunk = chunk // 128       # 16
    K = d + 1                        # 65: d rows of sh, row 64 = ones / t_sq

    nc = bacc.Bacc()
    s = nc.dram_tensor("s", [n_rows, d], F32, kind="ExternalInput")
    t = nc.dram_tensor("t", [q, d], F32, kind="ExternalInput")
    o = nc.dram_tensor("o", [n_rows, q], F32, kind="ExternalOutput")

    with tile.TileContext(nc) as tc:
        with (
            tc.tile_pool(name="const", bufs=1) as const,
            tc.tile_pool(name="work", bufs=4) as work,
            tc.tile_pool(name="chunks", bufs=4) as chunks,
            tc.tile_pool(name="stage", bufs=4) as stage,
            tc.tile_pool(name="psum_prep", bufs=3, space="PSUM") as psum_prep,
            tc.tile_pool(name="psum_pts", bufs=1, space="PSUM") as psum_pts,
            tc.tile_pool(name="psum_mm", bufs=2, space="PSUM") as psum_mm,
        ):
            S = const.tile([128, m_tiles * d], F32, name="S")

            identity = const.tile([128, 128], F32, name="identity")
            make_identity(nc, identity)
            neg2I = const.tile([128, 128], F32, name="neg2I")
            make_identity(nc, neg2I)
            nc.scalar.mul(neg2I, neg2I, -2.0)
            ones_mat = const.tile([d, 128], BF16, name="ones_mat")
            nc.vector.memset(ones_mat, 1.0)

            # PE warmup: dense fp32 matmuls to trip the HAM clock gate from
            # 4/8 (1.2 GHz) to 8/8 (2.4 GHz) early. The tiny DMA (on the
            # ACT ring, so the SP ring's input loads are not stalled) keeps
            # the chain live through DCE; the real output of that region is
            # written later (WAW-ordered).
            pw = psum_prep.tile([128, 128], F32, name="pw", tag="pp")
            for _ in range(9):
                nc.tensor.matmul(pw, identity, identity, start=True, stop=True)
            warm_sb = const.tile([1, 1], F32, name="warm_sb")
            nc.scalar.copy(warm_sb, pw[0:1, 0:1])
            nc.scalar.dma_start(out=o[0:1, 0:1], in_=warm_sb)

            A = const.tile([K, n_rows], BF16, name="A")   # sh rows 0..63, 64=ones
            B = const.tile([K, q], BF16, name="B")        # th rows 0..63, 64=t_sq
            s_sq = const.tile([128, m_tiles], F32, name="s_sq")
            nc.vector.memset(A[64:65, :], 1.0)

            # ---- s prep: per-tile transpose (PE) + fused square-reduce.
            # 4 transposes share one [64, 512] PSUM tile so the bf16
            # conversion is 1 big copy instead of 4 small ones. ----
            def prep_s():
                for m4 in range(m_tiles // 4):
                    pss = psum_prep.tile([d, 512], F32, name="pss", tag="pp")
                    for k in range(4):
                        m = m4 * 4 + k
                        V = S[:, m * d : (m + 1) * d]
                        # window k of pss = V.T @ (-2 I) = -2 s^T  (exact)
                        nc.tensor.matmul(
                            pss[:, k * 128 : (k + 1) * 128], V, neg2I,
                            start=True, stop=True,
                        )
                        # exact fp32 row sums of s^2 (native DVE ops -- the
                        # fused tensor_tensor_reduce is custom-ucode and not
                        # loadable in this runtime)
                        sqs = work.tile([128, d], F32, name="sqs", tag="sqs")
                        nc.vector.tensor_mul(sqs, V, V)
                        nc.vector.tensor_reduce(
                            s_sq[:, m : m + 1], sqs, mybir.AxisListType.X,
                            mybir.AluOpType.add,
                        )
                    dst = A[0:d, m4 * 512 : (m4 + 1) * 512]
                    if m4 % 2 == 0:
                        nc.scalar.copy(dst, pss)
                    else:
                        nc.vector.tensor_copy(dst, pss)

            # ---- t prep: the load and the engine processing are emitted
            # separately so loads can be queued far ahead ----
            g_tiles = {}

            def load_chunk(ch):
                base = ch * chunk
                # grouped layout: partition p holds t rows base+16p..+15,
                # giving 4KB contiguous runs (full DMA bus rate)
                G = chunks.tile([128, t_per_chunk * d], F32, name="G", tag="G")
                nc.sync.dma_start(
                    out=G[:, :].rearrange("p (j d) -> p j d", j=t_per_chunk, d=d),
                    in_=t[base : base + chunk, :].rearrange(
                        "(p j) d -> p j d", p=128, j=t_per_chunk
                    ),
                )
                g_tiles[ch] = G

            def transpose_chunk(ch):
                base = ch * chunk
                G = g_tiles.pop(ch)
                # B columns c = 16p + j: transpose view j, scatter stride 16.
                # All 16 transposes (3-deep PSUM rotation), with the
                # scatter-copy and a square of the just-written B columns
                # (in scatter order, so each square depends only on its own
                # quarter) interleaved on alternating engines.
                Bv = B[0:d, base : base + chunk].rearrange(
                    "e (p j) -> e j p", p=128, j=t_per_chunk
                )
                sqs4 = []
                for j4 in range(t_per_chunk // 4):
                    pst = psum_prep.tile([d, 512], F32, name="pst", tag="pp")
                    for k in range(4):
                        V = G[:, (j4 * 4 + k) * d : (j4 * 4 + k + 1) * d]
                        nc.tensor.transpose(
                            pst[:, k * 128 : (k + 1) * 128], V, identity
                        )
                    dst = Bv[:, j4 * 4 : (j4 + 1) * 4, :]
                    src = pst[:, :].rearrange("e (k p) -> e k p", k=4, p=128)
                    sqv = chunks.tile([d, 512], BF16, name="sq", tag="sq", bufs=4)
                    sqv3 = sqv[:, :].rearrange("e (k p) -> e k p", k=4, p=128)
                    if j4 % 2 == 0:
                        nc.scalar.copy(dst, src)
                        nc.vector.tensor_mul(sqv3, dst, dst)
                    else:
                        nc.vector.tensor_copy(dst, src)
                        nc.scalar.square(sqv3, dst)
                    sqs4.append(sqv)
                return sqs4

            def tsq_chunk(ch, sqs4):
                base = ch * chunk
                # t_sq = ones^T @ sq (bf16, 1 cycle/row), row 64 scattered
                # back with the same (k, p) pattern
                B64v = B[64:65, base : base + chunk].rearrange(
                    "e (p j) -> e j p", p=128, j=t_per_chunk
                )
                for j4, sqv in enumerate(sqs4):
                    pts = psum_pts.tile([128, 512], F32, name="pts", tag="pts")
                    nc.tensor.matmul(pts, ones_mat, sqv, start=True, stop=True)
                    tdst = B64v[:, j4 * 4 : (j4 + 1) * 4, :]
                    tsrc = pts[64:65, :].rearrange("e (k p) -> e k p", k=4, p=128)
                    if j4 % 2 == 0:
                        nc.scalar.copy(tdst, tsrc)
                    else:
                        nc.vector.tensor_copy(tdst, tsrc)

            def process_chunk(ch):
                tsq_chunk(ch, transpose_chunk(ch))

            # ---- main loop tile: one [128, chunk] staging tile.  PSUM
            # tiles span 2 banks (2 matmul windows) so each PSUM->SBUF
            # copy+bias is one [128, 1024] op; the two per tile alternate
            # ACT / DVE. ----
            # output rows of tile m are {16p + m}: affine partition stride
            ov = o[:, :].rearrange("(p m) q -> p m q", p=128, m=m_tiles)

            def main_tile(g, m, split_out=False):
                rows = slice(m * 128, (m + 1) * 128)
                stg = stage.tile([128, chunk], F32, name="stg", tag="stg")
                for ci in range(chunk // 1024):
                    ps = psum_mm.tile([128, 1024], F32, name="ps", tag="mm")
                    for k in range(2):
                        c = (g * chunk) // 512 + ci * 2 + k
                        cols = slice(c * 512, (c + 1) * 512)
                        nc.tensor.matmul(
                            ps[:, k * 512 : (k + 1) * 512], A[:, rows],
                            B[:, cols], start=True, stop=True,
                        )
                    dst = stg[:, ci * 1024 : (ci + 1) * 1024]
                    if (2 * m + ci) % 2 == 0:
                        nc.scalar.add(dst, ps, s_sq[:, m : m + 1])
                    else:
                        nc.vector.tensor_scalar_add(dst, ps, s_sq[:, m : m + 1])
                # alternate the two HWDGE rings (SP / ACT) for 2x the
                # DMA packet-processing throughput on the output stream
                out_eng = nc.sync if m % 2 == 0 else nc.scalar
                if not split_out:
                    out_eng.dma_start(
                        out=ov[:, m, g * chunk : (g + 1) * chunk], in_=stg
                    )
                else:
                    # last tile of the kernel: drain in two halves on both
                    # rings so the tail transfer is half as long
                    h = chunk // 2
                    nc.sync.dma_start(
                        out=ov[:, m, g * chunk : g * chunk + h], in_=stg[:, 0:h]
                    )
                    nc.scalar.dma_start(
                        out=ov[:, m, g * chunk + h : (g + 1) * chunk],
                        in_=stg[:, h:chunk],
                    )

            # software pipeline: the first 4 chunk loads are queued upfront
            # (G has 4 bufs); group g uses chunk g, whose engine processing
            # is emitted right after group g-1's tiles.  Only chunk 0 and
            # the s prep gate the first output DMA.
            # chunk 0 loads first so PE can transpose right after warmup;
            # s (needed later, by the first matmul/adds) follows it.  s uses
            # the same 16-rows-per-partition grouped layout (4KB runs):
            # partition p holds rows 16p..16p+15, so tile m covers rows
            # {16p + m}, and the output DMA addresses them with an affine
            # partition stride of 16 rows.
            load_chunk(0)
            nc.sync.dma_start(
                out=S[:, :].rearrange("p (m d) -> p m d", m=m_tiles, d=d),
                in_=s[:, :].rearrange("(p m) d -> p m d", p=128, m=m_tiles),
            )
            for ch in range(1, min(4, n_chunks)):
                load_chunk(ch)
            # chunk 0: s-prep slots between the transposes and the t_sq
            # matmuls, filling the PE stall while the squares land
            _sq0 = transpose_chunk(0)
            prep_s()
            tsq_chunk(0, _sq0)
            for g in range(n_chunks):
                for m in range(m_tiles):
                    # split the first tiles (smoother DMA ramp-up: each
                    # fused add releases its half immediately) and the very
                    # last one (halves the tail transfer)
                    split = (g == 0 and m < 2) or (
                        g == n_chunks - 1 and m == m_tiles - 1
                    )
                    main_tile(g, m, split_out=split)
                if g + 1 < n_chunks:
                    process_chunk(g + 1)
                if g + 4 < n_chunks:
                    load_chunk(g + 4)

    nc.finalize()
    return nc


_NC_CACHE = {}


def _get_nc(key=None):
    if key is None:
        key = (N_SHARD, Q, D)
    if key not in _NC_CACHE:
        _NC_CACHE[key] = build_nc(*key)
    return _NC_CACHE[key]


def make_in_maps(inputs):
    s = np.asarray(inputs["s"], dtype=np.float32)
    t = np.asarray(inputs["t"], dtype=np.float32)
    assert s.shape == (N, D) and t.shape == (Q, D), (s.shape, t.shape)
    return [{"s": s[c * N_SHARD : (c + 1) * N_SHARD], "t": t} for c in range(N_CORES)]


def _run(inputs, **spmd_kwargs):
    from concourse.bass_utils import run_bass_kernel_spmd

    nc = _get_nc()
    in_maps = make_in_maps(inputs)
    res = run_bass_kernel_spmd(nc, in_maps, list(range(N_CORES)), **spmd_kwargs)
    out = np.concatenate([res.results[c]["o"] for c in range(N_CORES)], axis=0)
    return out, res


def kernel(**inputs):
    out, _ = _run(inputs)
    return out


# revision 50
# speedup vs baseline: 1.2304x; 1.0054x over previous
"""Pairwise squared Euclidean distance on Trainium2, sharded over 8 NeuronCores.

dist[i, j] = ||s_i - t_j||^2 = s_sq[i] + t_sq[j] - 2 * (s @ t.T)[i, j]

Sharding: rows of s (and of the output) are split across the 8 cores;
t is replicated to every core. Each core computes a [2048, 16384] tile.

Per-core device program (single-matmul bf16 path):
  The tolerance (rel 2e-2) allows computing the cross term from the bf16
  hi parts alone: with A = bf16(-2*s)^T and B = bf16(t)^T, a single K=65
  matmul per output tile produces  -2*s@t.T + t_sq  in fp32 PSUM (row 64
  of A is all-ones against row 64 of B holding bf16(t_sq), computed on PE
  from an all-ones stationary operand over square(B)).  Measured rel err
  of this scheme is ~2e-3.  The exact fp32 per-partition s_sq[i] (DVE
  square + reduce) is added during the PSUM->SBUF copy (one fused
  [128, 1024] ACT-bias / DVE-tensor_scalar op per two PSUM banks), and
  staging tiles are DMA'd to the output on alternating HWDGE rings
  (SP / ACT).

  DMA traffic is minimized: both inputs are fetched with ONE strided DMA
  per 2048-row chunk in a 16-rows-per-partition grouped layout whose
  innermost contiguous run is 4KB (full DMA bus rate; <512B runs pay a
  2x penalty), then PE-transposed [128, 64] a column-view at a time;
  each transpose group lands in A/B via an affine stride-16 free-dim
  scatter that reconstructs global column order.  For s this means tile
  m covers rows {16p + m}, which the output DMA addresses with an affine
  partition stride of 16 rows -- same descriptor efficiency as row-major.
  t_sq is computed from each [64, 512] PSUM transpose group directly
  (square -> all-ones matmul -> row-64 scatter), so it never waits on a
  whole-chunk barrier.

  The main loop is emitted group-by-group (one 2048-column group per
  chunk); the first 4 chunk loads are queued upfront and chunk g+1's
  engine processing is emitted after group g's tiles, so the first
  output DMA is gated only on chunk 0's prep while Tile's range-accurate
  dependency tracking overlaps later prep with the saturated output
  stream.  The output is write-only traffic of 134 MB/core; the DMA
  model's bus rate (360 GB/s) puts the floor at ~373us, and the kernel
  sustains ~396us total (DMA busy ~386us, >97% occupancy).
"""

import numpy as np

import concourse.mybir as mybir
import concourse.tile as tile
from concourse import bacc
from concourse.masks import make_identity

F32 = mybir.dt.float32
BF16 = mybir.dt.bfloat16

N_CORES = 8
N, Q, D = 16384, 16384, 64
N_SHARD = N // N_CORES  # 2048


def build_nc(n_rows=N_SHARD, q=Q, d=D, chunk=2048):
    assert n_rows % 128 == 0 and q % chunk == 0
    assert chunk % 512 == 0 and d == 64
    m_tiles = n_rows // 128          # 16
    n_chunks = q // chunk            # 8
    t_per_chunk = chunk // 128       # 16
    K = d + 1                        # 65: d rows of sh, row 64 = ones / t_sq

    nc = bacc.Bacc()
    s = nc.dram_tensor("s", [n_rows, d], F32, kind="ExternalInput")
    t = nc.dram_tensor("t", [q, d], F32, kind="ExternalInput")
    o = nc.dram_tensor("o", [n_rows, q], F32, kind="ExternalOutput")

    with tile.TileContext(nc) as tc:
        with (
            tc.tile_pool(name="const", bufs=1) as const,
            tc.tile_pool(name="work", bufs=4) as work,
            tc.tile_pool(name="chunks", bufs=4) as chunks,
            tc.tile_pool(name="stage", bufs=4) as stage,
            tc.tile_pool(name="psum_prep", bufs=3, space="PSUM") as psum_prep,
            tc.tile_pool(name="psum_pts", bufs=1, space="PSUM") as psum_pts,
            tc.tile_pool(name="psum_mm", bufs=2, space="PSUM") as psum_mm,
        ):
            S = const.tile([128, m_tiles * d], F32, name="S")

            identity = const.tile([128, 128], F32, name="identity")
            make_identity(nc, identity)
            neg2I = const.tile([128, 128], F32, name="neg2I")
            make_identity(nc, neg2I)
            nc.scalar.mul(neg2I, neg2I, -2.0)
            ones_mat = const.tile([d, 128], BF16, name="ones_mat")
            nc.vector.memset(ones_mat, 1.0)

            # PE warmup: dense fp32 matmuls to trip the HAM clock gate from
            # 4/8 (1.2 GHz) to 8/8 (2.4 GHz) early. The tiny DMA (on the
            # ACT ring, so the SP ring's input loads are not stalled) keeps
            # the chain live through DCE; the real output of that region is
            # written later (WAW-ordered).
            pw = psum_prep.tile([128, 128], F32, name="pw", tag="pp")
            for _ in range(7):
                nc.tensor.matmul(pw, identity, identity, start=True, stop=True)
            warm_sb = const.tile([1, 1], F32, name="warm_sb")
            nc.scalar.copy(warm_sb, pw[0:1, 0:1])
            nc.scalar.dma_start(out=o[0:1, 0:1], in_=warm_sb)

            A = const.tile([K, n_rows], BF16, name="A")   # sh rows 0..63, 64=ones
            B = const.tile([K, q], BF16, name="B")        # th rows 0..63, 64=t_sq
            s_sq = const.tile([128, m_tiles], F32, name="s_sq")
            # single-partition memset is slow on DVE (1 lane); Pool runs
            # Memset at full efficiency and is otherwise idle
            nc.gpsimd.memset(A[64:65, :], 1.0)

            # ---- s prep: per-tile transpose (PE) + fused square-reduce.
            # 4 transposes share one [64, 512] PSUM tile so the bf16
            # conversion is 1 big copy instead of 4 small ones. ----
            def prep_A():
                for m4 in range(m_tiles // 4):
                    pss = psum_prep.tile([d, 512], F32, name="pss", tag="pp")
                    for k in range(4):
                        m = m4 * 4 + k
                        V = S[:, m * d : (m + 1) * d]
                        # window k of pss = V.T @ (-2 I) = -2 s^T (exact)
                        nc.tensor.matmul(
                            pss[:, k * 128 : (k + 1) * 128], V, neg2I,
                            start=True, stop=True,
                        )
                    dst = A[0:d, m4 * 512 : (m4 + 1) * 512]
                    if m4 % 2 == 0:
                        nc.scalar.copy(dst, pss)
                    else:
                        nc.vector.tensor_copy(dst, pss)

            def prep_ssq(m):
                # exact fp32 row sums of s^2 (native DVE ops -- the fused
                # tensor_tensor_reduce is custom-ucode and not loadable in
                # this runtime).  Emitted one tile ahead of its consumer in
                # group 0 so the 32 ops never pool up in front of the adds.
                V = S[:, m * d : (m + 1) * d]
                sqs = work.tile([128, d], F32, name="sqs", tag="sqs")
                nc.vector.tensor_mul(sqs, V, V)
                nc.vector.tensor_reduce(
                    s_sq[:, m : m + 1], sqs, mybir.AxisListType.X,
                    mybir.AluOpType.add,
                )

            # ---- t prep: the load and the engine processing are emitted
            # separately so loads can be queued far ahead ----
            g_tiles = {}

            def load_chunk(ch):
                base = ch * chunk
                # grouped layout: partition p holds t rows base+16p..+15,
                # giving 4KB contiguous runs (full DMA bus rate)
                G = chunks.tile([128, t_per_chunk * d], F32, name="G", tag="G")
                nc.sync.dma_start(
                    out=G[:, :].rearrange("p (j d) -> p j d", j=t_per_chunk, d=d),
                    in_=t[base : base + chunk, :].rearrange(
                        "(p j) d -> p j d", p=128, j=t_per_chunk
                    ),
                )
                g_tiles[ch] = G

            def transpose_chunk(ch):
                base = ch * chunk
                G = g_tiles.pop(ch)
                # B columns c = 16p + j: transpose view j, scatter stride 16.
                # All 16 transposes (3-deep PSUM rotation), with the
                # scatter-copy and a square of the just-written B columns
                # (in scatter order, so each square depends only on its own
                # quarter) interleaved on alternating engines.
                Bv = B[0:d, base : base + chunk].rearrange(
                    "e (p j) -> e j p", p=128, j=t_per_chunk
                )
                sqs4 = []
                for j4 in range(t_per_chunk // 4):
                    pst = psum_prep.tile([d, 512], F32, name="pst", tag="pp")
                    for k in range(4):
                        V = G[:, (j4 * 4 + k) * d : (j4 * 4 + k + 1) * d]
                        nc.tensor.transpose(
                            pst[:, k * 128 : (k + 1) * 128], V, identity
                        )
                    dst = Bv[:, j4 * 4 : (j4 + 1) * 4, :]
                    src = pst[:, :].rearrange("e (k p) -> e k p", k=4, p=128)
                    sqv = chunks.tile([d, 512], BF16, name="sq", tag="sq", bufs=4)
                    sqv3 = sqv[:, :].rearrange("e (k p) -> e k p", k=4, p=128)
                    if j4 % 2 == 0:
                        nc.scalar.copy(dst, src)
                        nc.vector.tensor_mul(sqv3, dst, dst)
                    else:
                        nc.vector.tensor_copy(dst, src)
                        nc.scalar.square(sqv3, dst)
                    sqs4.append(sqv)
                return sqs4

            def tsq_chunk(ch, sqs4):
                base = ch * chunk
                # t_sq = ones^T @ sq (bf16, 1 cycle/row), row 64 scattered
                # back with the same (k, p) pattern
                B64v = B[64:65, base : base + chunk].rearrange(
                    "e (p j) -> e j p", p=128, j=t_per_chunk
                )
                for j4, sqv in enumerate(sqs4):
                    pts = psum_pts.tile([128, 512], F32, name="pts", tag="pts")
                    nc.tensor.matmul(pts, ones_mat, sqv, start=True, stop=True)
                    tdst = B64v[:, j4 * 4 : (j4 + 1) * 4, :]
                    tsrc = pts[64:65, :].rearrange("e (k p) -> e k p", k=4, p=128)
                    if j4 % 2 == 0:
                        nc.scalar.copy(tdst, tsrc)
                    else:
                        nc.vector.tensor_copy(tdst, tsrc)

            def process_chunk(ch):
                tsq_chunk(ch, transpose_chunk(ch))

            # ---- main loop tile: one [128, chunk] staging tile.  PSUM
            # tiles span 2 banks (2 matmul windows) so each PSUM->SBUF
            # copy+bias is one [128, 1024] op; the two per tile alternate
            # ACT / DVE. ----
            # output rows of tile m are {16p + m}: affine partition stride
            ov = o[:, :].rearrange("(p m) q -> p m q", p=128, m=m_tiles)

            def main_tile(g, m, split_out=False):
                rows = slice(m * 128, (m + 1) * 128)
                stg = stage.tile([128, chunk], F32, name="stg", tag="stg")
                for ci in range(chunk // 1024):
                    ps = psum_mm.tile([128, 1024], F32, name="ps", tag="mm")
                    for k in range(2):
                        c = (g * chunk) // 512 + ci * 2 + k
                        cols = slice(c * 512, (c + 1) * 512)
                        nc.tensor.matmul(
                            ps[:, k * 512 : (k + 1) * 512], A[:, rows],
                            B[:, cols], start=True, stop=True,
                        )
                    dst = stg[:, ci * 1024 : (ci + 1) * 1024]
                    if (2 * m + ci) % 2 == 0:
                        nc.scalar.add(dst, ps, s_sq[:, m : m + 1])
                    else:
                        nc.vector.tensor_scalar_add(dst, ps, s_sq[:, m : m + 1])
                # alternate the two HWDGE rings (SP / ACT) for 2x the
                # DMA packet-processing throughput on the output stream
                out_eng = nc.sync if m % 2 == 0 else nc.scalar
                if not split_out:
                    out_eng.dma_start(
                        out=ov[:, m, g * chunk : (g + 1) * chunk], in_=stg
                    )
                else:
                    # last tile of the kernel: drain in two halves on both
                    # rings so the tail transfer is half as long
                    h = chunk // 2
                    nc.sync.dma_start(
                        out=ov[:, m, g * chunk : g * chunk + h], in_=stg[:, 0:h]
                    )
                    nc.scalar.dma_start(
                        out=ov[:, m, g * chunk + h : (g + 1) * chunk],
                        in_=stg[:, h:chunk],
                    )

            # software pipeline: the first 4 chunk loads are queued upfront
            # (G has 4 bufs); group g uses chunk g, whose engine processing
            # is emitted right after group g-1's tiles.  Only chunk 0 and
            # the s prep gate the first output DMA.
            # chunk 0 loads first so PE can transpose right after warmup;
            # s (needed later, by the first matmul/adds) follows it.  s uses
            # the same 16-rows-per-partition grouped layout (4KB runs):
            # partition p holds rows 16p..16p+15, so tile m covers rows
            # {16p + m}, and the output DMA addresses them with an affine
            # partition stride of 16 rows.
            load_chunk(0)
            nc.sync.dma_start(
                out=S[:, :].rearrange("p (m d) -> p m d", m=m_tiles, d=d),
                in_=s[:, :].rearrange("(p m) d -> p m d", p=128, m=m_tiles),
            )
            for ch in range(1, min(4, n_chunks)):
                load_chunk(ch)
            # chunk 0: s-prep slots between the transposes and the t_sq
            # matmuls, filling the PE stall while the squares land
            _sq0 = transpose_chunk(0)
            prep_A()
            tsq_chunk(0, _sq0)
            prep_ssq(0)
            prep_ssq(1)
            for g in range(n_chunks):
                for m in range(m_tiles):
                    if g == 0 and m + 2 < m_tiles:
                        prep_ssq(m + 2)
                    # split the first tiles (smoother DMA ramp-up: each
                    # fused add releases its half immediately) and the very
                    # last one (halves the tail transfer)
                    split = (g == 0 and m < 2) or (
                        g == n_chunks - 1 and m == m_tiles - 1
                    )
                    main_tile(g, m, split_out=split)
                if g + 1 < n_chunks:
                    process_chunk(g + 1)
                if g + 4 < n_chunks:
                    load_chunk(g + 4)

    nc.finalize()
    return nc


_NC_CACHE = {}


def _get_nc(key=None):
    if key is None:
        key = (N_SHARD, Q, D)
    if key not in _NC_CACHE:
        _NC_CACHE[key] = build_nc(*key)
    return _NC_CACHE[key]


def make_in_maps(inputs):
    s = np.asarray(inputs["s"], dtype=np.float32)
    t = np.asarray(inputs["t"], dtype=np.float32)
    assert s.shape == (N, D) and t.shape == (Q, D), (s.shape, t.shape)
    return [{"s": s[c * N_SHARD : (c + 1) * N_SHARD], "t": t} for c in range(N_CORES)]


def _run(inputs, **spmd_kwargs):
    from concourse.bass_utils import run_bass_kernel_spmd

    nc = _get_nc()
    in_maps = make_in_maps(inputs)
    res = run_bass_kernel_spmd(nc, in_maps, list(range(N_CORES)), **spmd_kwargs)
    out = np.concatenate([res.results[c]["o"] for c in range(N_CORES)], axis=0)
    return out, res


def kernel(**inputs):
    out, _ = _run(inputs)
    return out


# revision 51
# speedup vs baseline: 1.2396x; 1.0074x over previous
"""Pairwise squared Euclidean distance on Trainium2, sharded over 8 NeuronCores.

dist[i, j] = ||s_i - t_j||^2 = s_sq[i] + t_sq[j] - 2 * (s @ t.T)[i, j]

Sharding: rows of s (and of the output) are split across the 8 cores;
t is replicated to every core. Each core computes a [2048, 16384] tile.

Per-core device program (single-matmul bf16 path):
  The tolerance (rel 2e-2) allows computing the cross term from the bf16
  hi parts alone: with A = bf16(-2*s)^T and B = bf16(t)^T, a single K=65
  matmul per output tile produces  -2*s@t.T + t_sq  in fp32 PSUM (row 64
  of A is all-ones against row 64 of B holding bf16(t_sq), computed on PE
  from an all-ones stationary operand over square(B)).  Measured rel err
  of this scheme is ~2e-3.  The exact fp32 per-partition s_sq[i] (DVE
  square + reduce) is added during the PSUM->SBUF copy (one fused
  [128, 1024] ACT-bias / DVE-tensor_scalar op per two PSUM banks), and
  staging tiles are DMA'd to the output on alternating HWDGE rings
  (SP / ACT).

  DMA traffic is minimized: both inputs are fetched with ONE strided DMA
  per 2048-row chunk in a 16-rows-per-partition grouped layout whose
  innermost contiguous run is 4KB (full DMA bus rate; <512B runs pay a
  2x penalty), then PE-transposed [128, 64] a column-view at a time;
  each transpose group lands in A/B via an affine stride-16 free-dim
  scatter that reconstructs global column order.  For s this means tile
  m covers rows {16p + m}, which the output DMA addresses with an affine
  partition stride of 16 rows -- same descriptor efficiency as row-major.
  t_sq is computed from each [64, 512] PSUM transpose group directly
  (square -> all-ones matmul -> row-64 scatter), so it never waits on a
  whole-chunk barrier.

  The main loop is emitted group-by-group (one 2048-column group per
  chunk); the first 4 chunk loads are queued upfront and chunk g+1's
  engine processing is emitted after group g's tiles, so the first
  output DMA is gated only on chunk 0's prep while Tile's range-accurate
  dependency tracking overlaps later prep with the saturated output
  stream.  The output is write-only traffic of 134 MB/core; the DMA
  model's bus rate (360 GB/s) puts the floor at ~373us, and the kernel
  sustains ~396us total (DMA busy ~386us, >97% occupancy).
"""

import numpy as np

import concourse.mybir as mybir
import concourse.tile as tile
from concourse import bacc
from concourse.masks import make_identity

F32 = mybir.dt.float32
BF16 = mybir.dt.bfloat16

N_CORES = 8
N, Q, D = 16384, 16384, 64
N_SHARD = N // N_CORES  # 2048


def build_nc(n_rows=N_SHARD, q=Q, d=D, chunk=2048):
    assert n_rows % 128 == 0 and q % chunk == 0
    assert chunk % 512 == 0 and d == 64
    m_tiles = n_rows // 128          # 16
    n_chunks = q // chunk            # 8
    t_per_chunk = chunk // 128       # 16
    K = d + 1                        # 65: d rows of sh, row 64 = ones / t_sq

    nc = bacc.Bacc()
    s = nc.dram_tensor("s", [n_rows, d], F32, kind="ExternalInput")
    t = nc.dram_tensor("t", [q, d], F32, kind="ExternalInput")
    o = nc.dram_tensor("o", [n_rows, q], F32, kind="ExternalOutput")

    with tile.TileContext(nc) as tc:
        with (
            tc.tile_pool(name="const", bufs=1) as const,
            tc.tile_pool(name="work", bufs=4) as work,
            tc.tile_pool(name="chunks", bufs=4) as chunks,
            tc.tile_pool(name="stage", bufs=4) as stage,
            tc.tile_pool(name="psum_prep", bufs=3, space="PSUM") as psum_prep,
            tc.tile_pool(name="psum_pts", bufs=1, space="PSUM") as psum_pts,
            tc.tile_pool(name="psum_mm", bufs=2, space="PSUM") as psum_mm,
        ):
            S = const.tile([128, m_tiles * d], F32, name="S")

            identity = const.tile([128, 128], F32, name="identity")
            make_identity(nc, identity)
            neg2I = const.tile([128, 128], F32, name="neg2I")
            make_identity(nc, neg2I)
            nc.scalar.mul(neg2I, neg2I, -2.0)
            ones_mat = const.tile([d, 128], BF16, name="ones_mat")
            nc.vector.memset(ones_mat, 1.0)

            # PE warmup: dense fp32 matmuls to trip the HAM clock gate from
            # 4/8 (1.2 GHz) to 8/8 (2.4 GHz) early. The tiny DMA (on the
            # ACT ring, so the SP ring's input loads are not stalled) keeps
            # the chain live through DCE; the real output of that region is
            # written later (WAW-ordered).
            pw = psum_prep.tile([128, 128], F32, name="pw", tag="pp")
            for _ in range(7):
                nc.tensor.matmul(pw, identity, identity, start=True, stop=True)
            warm_sb = const.tile([1, 1], F32, name="warm_sb")
            nc.scalar.copy(warm_sb, pw[0:1, 0:1])
            nc.scalar.dma_start(out=o[0:1, 0:1], in_=warm_sb)

            A = const.tile([K, n_rows], BF16, name="A")   # sh rows 0..63, 64=ones
            B = const.tile([K, q], BF16, name="B")        # th rows 0..63, 64=t_sq
            s_sq = const.tile([128, m_tiles], F32, name="s_sq")
            # single-partition memset is slow on DVE (1 lane); Pool runs
            # Memset at full efficiency and is otherwise idle
            nc.gpsimd.memset(A[64:65, :], 1.0)

            # ---- s prep: per-tile transpose (PE) + fused square-reduce.
            # 4 transposes share one [64, 512] PSUM tile so the bf16
            # conversion is 1 big copy instead of 4 small ones. ----
            def prep_A():
                for m4 in range(m_tiles // 4):
                    pss = psum_prep.tile([d, 512], F32, name="pss", tag="pp")
                    for k in range(4):
                        m = m4 * 4 + k
                        V = S[:, m * d : (m + 1) * d]
                        # window k of pss = V.T @ (-2 I) = -2 s^T (exact)
                        nc.tensor.matmul(
                            pss[:, k * 128 : (k + 1) * 128], V, neg2I,
                            start=True, stop=True,
                        )
                    dst = A[0:d, m4 * 512 : (m4 + 1) * 512]
                    if m4 % 2 == 0:
                        nc.scalar.copy(dst, pss)
                    else:
                        nc.vector.tensor_copy(dst, pss)

            def prep_ssq(m):
                # exact fp32 row sums of s^2 (native DVE ops -- the fused
                # tensor_tensor_reduce is custom-ucode and not loadable in
                # this runtime).  Emitted one tile ahead of its consumer in
                # group 0 so the 32 ops never pool up in front of the adds.
                V = S[:, m * d : (m + 1) * d]
                sqs = work.tile([128, d], F32, name="sqs", tag="sqs")
                nc.vector.tensor_mul(sqs, V, V)
                nc.vector.tensor_reduce(
                    s_sq[:, m : m + 1], sqs, mybir.AxisListType.X,
                    mybir.AluOpType.add,
                )

            # ---- t prep: the load and the engine processing are emitted
            # separately so loads can be queued far ahead ----
            g_tiles = {}

            def load_chunk(ch):
                base = ch * chunk
                # grouped layout: partition p holds t rows base+16p..+15,
                # giving 4KB contiguous runs (full DMA bus rate)
                G = chunks.tile(
                    [128, t_per_chunk * d], F32, name="G", tag="G", bufs=8
                )
                nc.sync.dma_start(
                    out=G[:, :].rearrange("p (j d) -> p j d", j=t_per_chunk, d=d),
                    in_=t[base : base + chunk, :].rearrange(
                        "(p j) d -> p j d", p=128, j=t_per_chunk
                    ),
                )
                g_tiles[ch] = G

            def transpose_chunk(ch):
                base = ch * chunk
                G = g_tiles.pop(ch)
                # B columns c = 16p + j: transpose view j, scatter stride 16.
                # All 16 transposes (3-deep PSUM rotation), with the
                # scatter-copy and a square of the just-written B columns
                # (in scatter order, so each square depends only on its own
                # quarter) interleaved on alternating engines.
                Bv = B[0:d, base : base + chunk].rearrange(
                    "e (p j) -> e j p", p=128, j=t_per_chunk
                )
                sqs4 = []
                for j4 in range(t_per_chunk // 4):
                    pst = psum_prep.tile([d, 512], F32, name="pst", tag="pp")
                    for k in range(4):
                        V = G[:, (j4 * 4 + k) * d : (j4 * 4 + k + 1) * d]
                        nc.tensor.transpose(
                            pst[:, k * 128 : (k + 1) * 128], V, identity
                        )
                    dst = Bv[:, j4 * 4 : (j4 + 1) * 4, :]
                    src = pst[:, :].rearrange("e (k p) -> e k p", k=4, p=128)
                    sqv = chunks.tile([d, 512], BF16, name="sq", tag="sq", bufs=4)
                    sqv3 = sqv[:, :].rearrange("e (k p) -> e k p", k=4, p=128)
                    if j4 % 2 == 0:
                        nc.scalar.copy(dst, src)
                        nc.vector.tensor_mul(sqv3, dst, dst)
                    else:
                        nc.vector.tensor_copy(dst, src)
                        nc.scalar.square(sqv3, dst)
                    sqs4.append(sqv)
                return sqs4

            def tsq_chunk(ch, sqs4):
                base = ch * chunk
                # t_sq = ones^T @ sq (bf16, 1 cycle/row), row 64 scattered
                # back with the same (k, p) pattern
                B64v = B[64:65, base : base + chunk].rearrange(
                    "e (p j) -> e j p", p=128, j=t_per_chunk
                )
                for j4, sqv in enumerate(sqs4):
                    pts = psum_pts.tile([128, 512], F32, name="pts", tag="pts")
                    nc.tensor.matmul(pts, ones_mat, sqv, start=True, stop=True)
                    tdst = B64v[:, j4 * 4 : (j4 + 1) * 4, :]
                    tsrc = pts[64:65, :].rearrange("e (k p) -> e k p", k=4, p=128)
                    if j4 % 2 == 0:
                        nc.scalar.copy(tdst, tsrc)
                    else:
                        nc.vector.tensor_copy(tdst, tsrc)

            def process_chunk(ch):
                tsq_chunk(ch, transpose_chunk(ch))

            # ---- main loop tile: one [128, chunk] staging tile.  PSUM
            # tiles span 2 banks (2 matmul windows) so each PSUM->SBUF
            # copy+bias is one [128, 1024] op; the two per tile alternate
            # ACT / DVE. ----
            # output rows of tile m are {16p + m}: affine partition stride
            ov = o[:, :].rearrange("(p m) q -> p m q", p=128, m=m_tiles)

            def main_tile(g, m, split_out=False):
                rows = slice(m * 128, (m + 1) * 128)
                stg = stage.tile([128, chunk], F32, name="stg", tag="stg")
                for ci in range(chunk // 1024):
                    ps = psum_mm.tile([128, 1024], F32, name="ps", tag="mm")
                    for k in range(2):
                        c = (g * chunk) // 512 + ci * 2 + k
                        cols = slice(c * 512, (c + 1) * 512)
                        nc.tensor.matmul(
                            ps[:, k * 512 : (k + 1) * 512], A[:, rows],
                            B[:, cols], start=True, stop=True,
                        )
                    dst = stg[:, ci * 1024 : (ci + 1) * 1024]
                    if (2 * m + ci) % 2 == 0:
                        nc.scalar.add(dst, ps, s_sq[:, m : m + 1])
                    else:
                        nc.vector.tensor_scalar_add(dst, ps, s_sq[:, m : m + 1])
                # alternate the two HWDGE rings (SP / ACT) for 2x the
                # DMA packet-processing throughput on the output stream
                out_eng = nc.sync if m % 2 == 0 else nc.scalar
                if not split_out:
                    out_eng.dma_start(
                        out=ov[:, m, g * chunk : (g + 1) * chunk], in_=stg
                    )
                else:
                    # last tile of the kernel: drain in two halves on both
                    # rings so the tail transfer is half as long
                    h = chunk // 2
                    nc.sync.dma_start(
                        out=ov[:, m, g * chunk : g * chunk + h], in_=stg[:, 0:h]
                    )
                    nc.scalar.dma_start(
                        out=ov[:, m, g * chunk + h : (g + 1) * chunk],
                        in_=stg[:, h:chunk],
                    )

            # software pipeline: the first 4 chunk loads are queued upfront
            # (G has 4 bufs); group g uses chunk g, whose engine processing
            # is emitted right after group g-1's tiles.  Only chunk 0 and
            # the s prep gate the first output DMA.
            # chunk 0 loads first so PE can transpose right after warmup;
            # s (needed later, by the first matmul/adds) follows it.  s uses
            # the same 16-rows-per-partition grouped layout (4KB runs):
            # partition p holds rows 16p..16p+15, so tile m covers rows
            # {16p + m}, and the output DMA addresses them with an affine
            # partition stride of 16 rows.  ALL chunk loads are queued
            # upfront (G has 8 bufs): their ~13us of transfers exactly fill
            # the DMA-idle prep window, so the saturated output stream
            # later never shares the bus with input traffic.
            load_chunk(0)
            nc.sync.dma_start(
                out=S[:, :].rearrange("p (m d) -> p m d", m=m_tiles, d=d),
                in_=s[:, :].rearrange("(p m) d -> p m d", p=128, m=m_tiles),
            )
            for ch in range(1, n_chunks):
                load_chunk(ch)
            # chunk 0: s-prep slots between the transposes and the t_sq
            # matmuls, filling the PE stall while the squares land
            _sq0 = transpose_chunk(0)
            prep_A()
            tsq_chunk(0, _sq0)
            prep_ssq(0)
            prep_ssq(1)
            for g in range(n_chunks):
                for m in range(m_tiles):
                    if g == 0 and m + 2 < m_tiles:
                        prep_ssq(m + 2)
                    # split the first tiles (smoother DMA ramp-up: each
                    # fused add releases its half immediately) and the very
                    # last one (halves the tail transfer)
                    split = (g == 0 and m < 2) or (
                        g == n_chunks - 1 and m == m_tiles - 1
                    )
                    main_tile(g, m, split_out=split)
                if g + 1 < n_chunks:
                    process_chunk(g + 1)

    nc.finalize()
    return nc


_NC_CACHE = {}


def _get_nc(key=None):
    if key is None:
        key = (N_SHARD, Q, D)
    if key not in _NC_CACHE:
        _NC_CACHE[key] = build_nc(*key)
    return _NC_CACHE[key]


def make_in_maps(inputs):
    s = np.asarray(inputs["s"], dtype=np.float32)
    t = np.asarray(inputs["t"], dtype=np.float32)
    assert s.shape == (N, D) and t.shape == (Q, D), (s.shape, t.shape)
    return [{"s": s[c * N_SHARD : (c + 1) * N_SHARD], "t": t} for c in range(N_CORES)]


def _run(inputs, **spmd_kwargs):
    from concourse.bass_utils import run_bass_kernel_spmd

    nc = _get_nc()
    in_maps = make_in_maps(inputs)
    res = run_bass_kernel_spmd(nc, in_maps, list(range(N_CORES)), **spmd_kwargs)
    out = np.concatenate([res.results[c]["o"] for c in range(N_CORES)], axis=0)
    return out, res


def kernel(**inputs):
    out, _ = _run(inputs)
    return out
